# revision 28
# baseline (speedup 1.0000x reference)
"""CapsuleLayer (dynamic routing) Trainium2 Bass kernel.

Sharding: pure data-parallel over batch B=256 -> 8 cores x 32 batches.
Per core the 32 batches run as 4 sub-chunks of 8; the SBUF partition dim
packs p = b*16 + ig where capsule index i = 16*g + ig, g in [0,72).

Phase 1 (u_hat = einsum('nidk,bik->bnid')): the K=8 contraction is packed
to K=128 by block-diagonalizing 16 capsules' inputs into the PE stationary
operand. The block-diagonal values arrive PRE-MASKED from the host (zeros
off the 4-wide near-diagonal), so on-chip assembly is 4 strided DVE copies
per sub-chunk into a once-memset buffer (far-off-diagonal zeros persist):
    lhsT[(ig,k), (b',ig')] = x[b', 16g+ig', k] * (ig==ig')
    rhs  = W2[(ig,k), (g,d,n)] = W[n, 16g+ig, d, k]
    psum[(b,ig), (d,n)] = u_hat[b, n, 16g+ig, d]
u_hat stays on-chip in bf16 as U[128, G, D, N] (n innermost so broadcast
multiplies keep unit stride for the DVE 2x mode). PSUM->SBUF evacuation is
batched 3 g per copy and split across DVE/ACT/Pool.

Routing (3 iters, fused on-chip):
  outputs-einsum: DVE multiply (c bcast over d) + PE partition-reduce with a
  fixed 0/1 bmask stationary accumulating all 72 groups into one PSUM bank
  (the r=0 uniform softmax weight 1/N is folded into a scaled mask).
  agreement: DVE multiply (outputs bcast over g) + fold-tree d-reduction:
  levels 1-2 on DVE (bf16 2x), levels 3-4 + bl accumulate on GPSIMD.
  softmax over n: ACT exp; the Z reduce and the e*(1/Z) multiply on GPSIMD
  (logits bounded, no max-sub).

Emission is step-major (lockstep) across the 4 sub-chunks: engines execute
their streams in order, so per-sub-chunk emission would serialize the whole
routing chain; lockstep keeps each engine's queue dense.
"""

import numpy as np
import ml_dtypes

B, N, I, D, DK = 256, 10, 1152, 16, 8
NCORES = 8
BC = B // NCORES      # 32 batches per core
BS = 8                # batches per sub-chunk
NSUB = BC // BS       # 4
IG = 16               # capsules per PE group
G = I // IG           # 72
ND = D * N            # 160, (d-major, n-minor)
GBLK = 12             # g per routing block
NBLK = G // GBLK      # 6
CB = 3                # g per PSUM evacuation copy
XSL = G * BS * 4      # per-sub-chunk xc slice (pre-masked values)
BF16 = ml_dtypes.bfloat16

_cache = {}


def _bcast(ap, axis, count):
    """Insert a stride-0 dim of size `count` at `axis`."""
    ap = ap.unsqueeze(axis)
    shape = list(ap.shape)
    shape[axis] = count
    return ap.broadcast_to(shape)


def _legalize_waits(nc):
    """This walrus build takes at most 1 embedded sync wait per TPB
    instruction (2 on EventSemaphore, 0 on Drain). Tile emits multi-wait
    sync_info; hoist the extras onto preceding EventSemaphore instructions
    on the same engine queue."""
    from concourse import mybir

    n = 0
    for fn in nc.m.functions:
        for blk in fn.blocks:
            out = []
            for inst in blk.instructions:
                si = inst.sync_info
                if si is not None and si.on_wait:
                    keep = 1
                    if inst.opcode == "Drain":
                        keep = 0
                    elif inst.opcode == "EventSemaphore":
                        keep = 2
                    w = list(si.on_wait)
                    if len(w) > keep:
                        extra = w[:len(w) - keep] if keep else w
                        kept = w[len(w) - keep:] if keep else []
                        for i0 in range(0, len(extra), 2):
                            n += 1
                            out.append(mybir.InstEventSemaphore(
                                name=f"{inst.name}-hw{n}",
                                engine=inst.engine, ins=[], outs=[],
                                sync_info=mybir.SyncInfo(
                                    on_wait=extra[i0:i0 + 2],
                                    on_update=[]),
                            ))
                        si.on_wait = kept
                out.append(inst)
            blk.instructions = out
    return n


def _build_nc():
    import concourse.bass as bass
    import concourse.tile as tile
    from concourse import mybir
    from contextlib import ExitStack

    f32 = mybir.dt.float32
    bf16 = mybir.dt.bfloat16
    AX = mybir.AxisListType
    OP = mybir.AluOpType
    AF = mybir.ActivationFunctionType

    nc = bass.Bass()
    xc_d = nc.dram_tensor("xc", [128, NSUB, XSL], bf16, kind="ExternalInput")
    w2_d = nc.dram_tensor("w2", [128, G * ND], bf16, kind="ExternalInput")
    bmask_d = nc.dram_tensor("bmask", [128, 2, BS], bf16,
                             kind="ExternalInput")
    bcmask_d = nc.dram_tensor("bcmask", [BS, 128], bf16, kind="ExternalInput")
    y_d = nc.dram_tensor("y", [NSUB, BS, ND], f32, kind="ExternalOutput")

    with tile.TileContext(nc) as tc:
        with ExitStack() as ctx:
            singles = ctx.enter_context(tc.tile_pool(name="singles", bufs=1))
            upool = ctx.enter_context(tc.tile_pool(name="upool", bufs=4))

            GQ = G // 4
            Us = []
            ps_os0 = {}
            outps = ctx.enter_context(
                tc.tile_pool(name="outps", bufs=4, space="PSUM"))
            # ---------------- Phase A: u_hat build ----------------
            with ExitStack() as actx:
                xcpool = actx.enter_context(
                    tc.tile_pool(name="xcpool", bufs=1))
                ph1ps = actx.enter_context(
                    tc.tile_pool(name="ph1ps", bufs=4, space="PSUM"))

                # interleave xc / w2 quarter loads: xc[s] gates the s-th
                # sub-chunk's stationary build, w2q[q] the matmuls
                xcs = [None] * 4
                w2q = [None] * 4
                for s in range(NSUB):
                    xct = xcpool.tile([128, XSL], bf16, tag=f"xc{s}")
                    nc.sync.dma_start(xct, xc_d[:, s])
                    xcs[s] = xct
                    w2t = xcpool.tile([128, GQ * ND], bf16, tag=f"w2_{s}")
                    nc.sync.dma_start(
                        w2t, w2_d[:, s * GQ * ND:(s + 1) * GQ * ND])
                    w2q[s] = w2t
                bmasks = singles.tile([128, 2, BS], bf16)
                nc.sync.dma_start(bmasks, bmask_d[:])
                bmask = bmasks[:, 0]    # 1.0 diag-expand mask
                bmask0 = bmasks[:, 1]   # 1/N-scaled mask for the r=0 mean
                bcmask = singles.tile([BS, 128], bf16)
                nc.sync.dma_start(bcmask, bcmask_d[:])

                # the buffer clears gate the stationary builds: split
                # each DVE/Pool (Pool is slightly faster per element)
                GSP = 40  # Pool's share of the 72 g-groups
                xblk0 = xcpool.tile([128, G, BS, IG], bf16, tag="xblk0")
                nc.gpsimd.memset(xblk0[:, :GSP], 0.0)
                nc.vector.memset(xblk0[:, GSP:], 0.0)
                xblk1 = xcpool.tile([128, G, BS, IG], bf16, tag="xblk1")
                nc.gpsimd.memset(xblk1[:, :GSP], 0.0)
                nc.vector.memset(xblk1[:, GSP:], 0.0)
                xblks = [xblk0, xblk1, xblk0, xblk1]

                def build_diag(s):
                    xb, xv = xblks[s], xcs[s]
                    for j in range(4):
                        xvj = bass.AP(
                            tensor=xv.tensor, offset=xv.offset,
                            ap=list(xv.ap[:1]) + [[BS * 4, G], [4, BS],
                                                  [1, 4]],
                        )[32 * j:32 * (j + 1)]
                        nc.vector.tensor_copy(
                            xb[32 * j:32 * (j + 1), :, :,
                               4 * j:4 * (j + 1)], xvj)
                for s in range(2):
                    build_diag(s)

                # engine rotation for the psum->sbuf copies (CB=6 g each);
                # GPSIMD cannot read PSUM; split ACT 8 : DVE 4
                def ucopy(dst, src, k):
                    if k % 12 in (1, 3, 6, 8, 10):
                        nc.vector.tensor_copy(dst, src)
                    else:
                        nc.scalar.copy(dst, src)

                for s in range(NSUB):
                    if s >= 2:
                        build_diag(s)
                    xb = xblks[s]
                    U = upool.tile([128, G, D, N], bf16, tag="U")
                    Us.append(U)
                    for kc in range(G // CB):
                        ps = ph1ps.tile([128, CB, D, N], f32, tag="ph1")
                        for q in range(CB):
                            g = kc * CB + q
                            qq, gq = g // GQ, g % GQ
                            nc.tensor.matmul(
                                ps[:, q], xb[:, g],
                                w2q[qq][:, gq * ND:(gq + 1) * ND],
                                start=True, stop=True)
                        ucopy(U[:, kc * CB:(kc + 1) * CB], ps,
                              s * (G // CB) + kc)
                    # r=0 i-sum rides right behind this sub-chunk's
                    # evacuation on the PE queue (subtile deps on U)
                    ps_os0[s] = outps.tile([BS, D, N], f32, tag="po",
                                           name=f"po0_{s}")
                    for j in range(G):
                        nc.tensor.matmul(
                            ps_os0[s], bmask0, Us[s][:, j],
                            start=(j == 0), stop=(j == G - 1),
                            skip_group_check=True)

            # ---------------- Routing pools ----------------
            tpool = ctx.enter_context(tc.tile_pool(name="tpool", bufs=1))
            tfpool = ctx.enter_context(tc.tile_pool(name="tfpool", bufs=1))
            tmpool = ctx.enter_context(tc.tile_pool(name="tmpool", bufs=1))
            blpool = ctx.enter_context(tc.tile_pool(name="blpool", bufs=4))
            smpool = ctx.enter_context(tc.tile_pool(name="smpool", bufs=2))
            cpool = ctx.enter_context(tc.tile_pool(name="cpool", bufs=4))
            obcpool = ctx.enter_context(tc.tile_pool(name="obc", bufs=4))
            tiny = ctx.enter_context(tc.tile_pool(name="tiny", bufs=1))
            bcps = ctx.enter_context(
                tc.tile_pool(name="bcps", bufs=2, space="PSUM"))

            bls = [blpool.tile([128, G, N], f32, tag="bl", name=f"bl{s}")
                   for s in range(NSUB)]

            # --- per-sub-chunk squash stages (emitted interleaved) ---
            sqst = {}

            def sq_v(s, r, ps):
                """DVE psum evacuation + ACT square (autonomous after)."""
                v = tiny.tile([BS, D, N], f32, tag=f"v{s}", name=f"v{r}{s}")
                nc.vector.tensor_copy(v, ps)
                vsq = tiny.tile([BS, D, N], f32, tag=f"vsq{s}",
                                name=f"vsq{r}{s}")
                nc.scalar.square(vsq, v)
                sqst[(r, s)] = (v, vsq)

            def sq_chain(s, r):
                """DVE nsq..ov (+ACT sqrt); r==2 DMAs the result out."""
                v, vsq = sqst[(r, s)]
                nsq = tiny.tile([BS, N], f32, tag=f"ns{s}", name=f"ns{r}{s}")
                nc.vector.tensor_reduce(
                    nsq, vsq.transpose([0, 2, 1]), axis=AX.X, op=OP.add)
                sq = tiny.tile([BS, N], f32, tag=f"sq{s}", name=f"sq{r}{s}")
                nc.scalar.sqrt(sq, nsq)
                # t1 = (nsq + 1) * sq  == |v|^3 + |v|
                t1 = tiny.tile([BS, N], f32, tag=f"t1{s}", name=f"t1{r}{s}")
                nc.vector.scalar_tensor_tensor(
                    t1, nsq, 1.0, sq, op0=OP.add, op1=OP.mult)
                rec = tiny.tile([BS, N], f32, tag=f"rec{s}",
                                name=f"rec{r}{s}")
                nc.vector.reciprocal(rec, t1)
                fac = tiny.tile([BS, N], f32, tag=f"fac{s}",
                                name=f"fac{r}{s}")
                nc.vector.tensor_mul(fac, nsq, rec)
                ov = tiny.tile([BS, D, N], f32, tag=f"ov{s}",
                               name=f"ov{r}{s}")
                nc.vector.tensor_mul(ov, v, _bcast(fac, 1, D))
                if r == 2:
                    nc.sync.dma_start(y_d[s], ov)
                    return None
                ob = tiny.tile([BS, D, N], bf16, tag=f"ob{s}",
                               name=f"ob{r}{s}")
                nc.vector.tensor_copy(ob, ov)
                sqst[(r, s, "ob")] = ob
                return ob

            def sq_psb(s, r):
                """PE broadcast matmul [BS,...] -> [128,...]."""
                psb = bcps.tile([128, D, N], f32, tag="bc", name=f"bc{r}{s}")
                nc.tensor.matmul(psb, bcmask, sqst[(r, s, "ob")],
                                 start=True, stop=True)
                sqst[(r, s, "psb")] = psb

            def sq_obc(s, r):
                """DVE psum -> sbuf bf16 broadcast result."""
                obc = obcpool.tile([128, D, N], bf16, tag="obc",
                                   name=f"obc{r}{s}")
                nc.vector.tensor_copy(obc, sqst[(r, s, "psb")])
                return obc

            AGRW = 8   # (blk, s) pairs in flight per agreement window

            def agreement_all(obcs, first, hook=None):
                """b_l[s] (+)= sum_d U[s] * obc[s], level-major in windows
                so each engine's in-order dispatch never stalls on its own
                chain. Fold levels 1-2 on DVE (bf16 2x), 3-4 + accumulate
                on GPSIMD. Pairs are s-major so early sub-chunks' logits
                finalize first; `hook` (emitted after the second window,
                by which point s0/s1 accumulates are fully emitted) starts
                the next round's s0 work inside this round's stream."""
                pairs = [(blk, s) for s in range(NSUB)
                         for blk in range(NBLK)]
                for wi, w0 in enumerate(range(0, len(pairs), AGRW)):
                    win = pairs[w0:w0 + AGRW]
                    t2s, t2fs = {}, {}
                    for blk, s in win:
                        g0 = blk * GBLK
                        ri = (s * NBLK + blk) % AGRW
                        t2 = tpool.tile([128, GBLK, D, N], bf16,
                                        tag=f"t2_{ri}",
                                        name=f"t2_{w0}_{blk}_{s}")
                        nc.vector.tensor_mul(
                            t2, Us[s][:, g0:g0 + GBLK],
                            _bcast(obcs[s], 1, GBLK))
                        t2s[(blk, s)] = t2
                    for blk, s in win:
                        t2 = t2s[(blk, s)]
                        ri = (s * NBLK + blk) % AGRW
                        t2f = tfpool.tile([128, GBLK, 8, N], bf16,
                                          tag=f"t2f_{ri}",
                                          name=f"t2f_{w0}_{blk}_{s}")
                        nc.vector.tensor_add(
                            t2f, t2[:, :, 0:8], t2[:, :, 8:16])
                        t2fs[(blk, s)] = t2f
                    for blk, s in win:
                        t2f = t2fs[(blk, s)]
                        nc.vector.tensor_add(
                            t2f[:, :, 0:4], t2f[:, :, 0:4], t2f[:, :, 4:8])
                    for blk, s in win:
                        t2f = t2fs[(blk, s)]
                        nc.gpsimd.tensor_add(
                            t2f[:, :, 0:2], t2f[:, :, 0:2], t2f[:, :, 2:4])
                    for blk, s in win:
                        t2f = t2fs[(blk, s)]
                        g0 = blk * GBLK
                        if first:
                            nc.gpsimd.tensor_add(
                                bls[s][:, g0:g0 + GBLK],
                                t2f[:, :, 0], t2f[:, :, 1])
                        else:
                            nc.gpsimd.tensor_add(
                                t2f[:, :, 0], t2f[:, :, 0], t2f[:, :, 1])
                            nc.gpsimd.tensor_add(
                                bls[s][:, g0:g0 + GBLK],
                                bls[s][:, g0:g0 + GBLK], t2f[:, :, 0])
                    if hook is not None and wi == 1:
                        hook()

            # --- round machinery, keyed (r, s) so a round's s0 work can
            # start inside the previous round's agreement stream ---
            es, cs, tms, ps_os = {}, {}, {}, {}

            def emit_exp(r, s):
                es[(r, s)] = smpool.tile([128, G, N], f32, tag="e",
                                         name=f"e{r}{s}")
                nc.scalar.activation(es[(r, s)], bls[s], AF.Exp)

            def softmax_tail(r, s):
                """DVE z-reduce + recip; c-mult on DVE for s=0 (so the
                first tm never waits on Pool's fold tail), Pool after."""
                z = smpool.tile([128, G], f32, tag="z", name=f"z{r}{s}")
                nc.vector.tensor_reduce(z, es[(r, s)], axis=AX.X, op=OP.add)
                rz = smpool.tile([128, G], f32, tag="rz", name=f"rz{r}{s}")
                nc.vector.reciprocal(rz, z)
                c = cpool.tile([128, G, N], bf16, tag="c", name=f"c{r}{s}")
                eng = nc.vector if s <= 1 else nc.gpsimd
                eng.tensor_mul(c, es[(r, s)], _bcast(rz, 2, N))
                cs[(r, s)] = c

            def emit_tm(r, s):
                out = []
                for blk in range(NBLK):
                    g0 = blk * GBLK
                    tm = tmpool.tile([128, GBLK, D, N], bf16,
                                     tag=f"tm_{blk}",
                                     name=f"tm{r}_{s}_{blk}")
                    nc.vector.tensor_mul(
                        tm, Us[s][:, g0:g0 + GBLK],
                        _bcast(cs[(r, s)][:, g0:g0 + GBLK], 2, D))
                    out.append(tm)
                tms[(r, s)] = out

            def emit_isum(r, s):
                tml = tms[(r, s)]
                ps_os[(r, s)] = outps.tile([BS, D, N], f32, tag="po",
                                           name=f"po{r}_{s}")
                for j in range(G):
                    nc.tensor.matmul(
                        ps_os[(r, s)], bmask, tml[j // GBLK][:, j % GBLK],
                        start=(j == 0), stop=(j == G - 1),
                        skip_group_check=True)

            def round_start(r):
                """Emitted from inside the previous round's agreement (after
                its second window): s0's softmax, weighting and i-sum run
                while the agreement tail is still in flight."""
                emit_exp(r, 0)
                emit_exp(r, 1)
                softmax_tail(r, 0)
                emit_tm(r, 0)
                emit_isum(r, 0)

            def round_rest(r):
                # s0's squash chain first: its ACT ops (square/sqrt) must
                # not queue behind exp(s2/s3), which wait on the agreement
                # tail's Pool accumulates
                sq_v(0, r, ps_os[(r, 0)])
                sq_chain(0, r)
                softmax_tail(r, 1)
                emit_tm(r, 1)
                emit_isum(r, 1)
                emit_exp(r, 2)
                softmax_tail(r, 2)
                emit_tm(r, 2)
                if r < 2:
                    sq_psb(0, r)  # after isum(1) on the PE queue
                sq_v(1, r, ps_os[(r, 1)])
                sq_chain(1, r)
                emit_exp(r, 3)
                softmax_tail(r, 3)
                emit_tm(r, 3)
                emit_isum(r, 2)
                sq_v(2, r, ps_os[(r, 2)])
                sq_chain(2, r)
                emit_isum(r, 3)
                if r < 2:
                    sq_psb(1, r)
                    sq_psb(2, r)
                sq_v(3, r, ps_os[(r, 3)])
                sq_chain(3, r)
                if r < 2:
                    sq_psb(3, r)
                    return {s: sq_obc(s, r) for s in range(NSUB)}
                return None

            # ---- r=0: i-sum already accumulated during phase A;
            # squash per s (psums long done by phase-A end), psb rides
            # the idle PE right after the phase-A/i-sum stream ----
            for s in range(NSUB):
                sq_v(s, 0, ps_os0[s])
            for s in range(NSUB):
                sq_chain(s, 0)
                sq_psb(s, 0)
            obcs0 = {s: sq_obc(s, 0) for s in range(NSUB)}
            agreement_all(obcs0, first=True, hook=lambda: round_start(1))
            obcs1 = round_rest(1)
            agreement_all(obcs1, first=False, hook=lambda: round_start(2))
            round_rest(2)
    _legalize_waits(nc)
    return nc


def _prep_inputs(inputs, W):
    """Host-side layout prep. Returns per-core input maps."""
    W = np.asarray(W, dtype=np.float32)
    inputs = np.asarray(inputs, dtype=np.float32)
    # W2[(ig,k), (g,d,n)] = W[n, 16g+ig, d, k]
    Wr = W.reshape(N, G, IG, D, DK)
    w2 = np.ascontiguousarray(
        Wr.transpose(2, 4, 1, 3, 0)).reshape(128, G * ND).astype(BF16)
    bm = np.repeat(np.eye(BS, dtype=np.float32), IG, axis=0)
    bmask = np.stack([bm, bm / N], axis=1).astype(BF16)  # [128, 2, BS]
    bcmask = np.repeat(np.eye(BS, dtype=np.float32), IG, axis=1).astype(BF16)

    # mask over the 4-wide near-diagonal: keep[(ig,k), t] = (ig%4==t)
    ig_idx = np.arange(128) // DK          # ig of partition (ig,k)
    keep = (ig_idx[:, None] % 4 == np.arange(4)[None, :]).astype(np.float32)

    in_maps = []
    for cc in range(NCORES):
        xcore = inputs[cc * BC:(cc + 1) * BC]       # [32, 1152, 8]
        xr = xcore.reshape(NSUB, BS, G, IG, DK)     # [s, b, g, ig, k]
        # xq[(ig,k), s, g, b, t] = x[s*8+b, 16g + 4*(ig//4) + t, k]
        # pre-masked: zero where ig%4 != t, so the on-chip strided copy
        # lands exactly the block-diagonal values
        xq = np.empty((IG, DK, NSUB, G, BS, 4), np.float32)
        for t in range(4):
            v = xr[:, :, :, t::4, :]                # [s, b, g, ig4=4, k]
            xq[:, :, :, :, :, t] = np.repeat(
                v.transpose(3, 4, 0, 2, 1), 4, axis=0)  # [ig, k, s, g, b]
        xq *= keep[:, None, None, None, :].reshape(IG, DK, 1, 1, 1, 4)
        xq = xq.reshape(128, NSUB, XSL).astype(BF16)
        in_maps.append(
            {"xc": xq, "w2": w2, "bmask": bmask, "bcmask": bcmask})
    return in_maps


def _run(inputs, W, trace=False):
    from concourse.bass_utils import run_bass_kernel_spmd

    if "nc" not in _cache:
        _cache["nc"] = _build_nc()
    nc = _cache["nc"]
    in_maps = _prep_inputs(inputs, W)
    res = run_bass_kernel_spmd(
        nc, in_maps, core_ids=list(range(NCORES)), trace=trace)
    # y[s, b, (d, n)] per core -> out[b_global, n, d]
    out = np.empty((B, N, D), np.float32)
    for cc in range(NCORES):
        yc = res.results[cc]["y"].reshape(NSUB, BS, D, N)
        out[cc * BC:(cc + 1) * BC] = yc.transpose(0, 1, 3, 2).reshape(
            BC, N, D)
    return out, res


def kernel(inputs, W):
    out, _ = _run(inputs, W, trace=False)
    return out
